# revision 1
# baseline (speedup 1.0000x reference)
"""Cached self-attention Trainium2 kernel (v3).

Sharding: 8 cores = 2 batches x 4 head-groups. Core c: batch b=c//4, group
g=c%4 owns heads 4g..4g+3 (columns 512g:512g+512 of the q/k/v projections).
Each core projects q/k/v for its heads over the full sequence, runs attention
for its 4 heads, the 4 cores of a batch AllGather the (normalized, transposed)
per-head attention outputs, and each core computes the output projection onto
its 512-column slice of wo (full sequence), so outputs tile the model dim.

All matmuls fp16 x fp16 -> fp32 PSUM. Softmax: exp on ScalarE with the
1/sqrt(128) scale folded into the q evacuation; Z via fp16 DVE running adds +
ones-matmul cross-partition sum + fast reciprocal; normalization applied
during PSUM evacuation of the attention output.
"""
import numpy as np
from contextlib import ExitStack

import concourse.bass as bass
import concourse.tile as tile
from concourse import bacc, mybir
from concourse.bass_utils import run_bass_kernel_spmd

B, S, PC, D, H = 2, 2048, 2048, 2048, 16
HD = D // H            # 128 head dim
GH = H // 4            # 4 heads per core
DG = GH * HD           # 512 head-dims per core
NB = 512               # block size
NKC = (PC + S) // HD   # 32 key chunks of 128
NDC = D // HD          # 16 contraction chunks
F16 = mybir.dt.float16
F32 = mybir.dt.float32
AF = mybir.ActivationFunctionType
ALU = mybir.AluOpType
INV_SQRT_HD = float(1.0 / np.sqrt(HD))

GROUPS = [[0, 1, 2, 3], [4, 5, 6, 7]]


def build():
    nc = bacc.Bacc("TRN2", target_bir_lowering=False, debug=False, num_devices=8)

    def inp(name, shape):
        return nc.dram_tensor(name, shape, F16, kind="ExternalInput").ap()

    xT = inp("xT", [D, S])          # x[b].T
    wq = inp("wq", [D, DG])         # wq[:, 512g:512g+512]
    bq = inp("bq", [DG])            # bq slice / sqrt(HD)
    wk = inp("wk", [D, DG])
    bk = inp("bk", [DG])
    wv = inp("wv", [D, DG])
    bv = inp("bv", [DG])
    ckT = inp("ckT", [DG, PC])      # cache_k[b,:,slice].T
    cv = inp("cv", [PC, DG])        # cache_v[b,:,slice]
    wo = inp("wo", [D, DG])         # wo rows permuted to gather order, cols sliced
    bo = inp("bo", [DG])
    y = nc.dram_tensor("y", [S, DG], F32, kind="ExternalOutput").ap()

    with tile.TileContext(nc) as tc, ExitStack() as ctx:
        res = ctx.enter_context(tc.tile_pool(name="res", bufs=1))
        dram = ctx.enter_context(tc.tile_pool(name="dram", bufs=1, space="DRAM"))

        # tiny whole-kernel residents
        bq_t = res.tile([HD, GH], F16, tag="bq")
        bk_t = res.tile([HD, GH], F16, tag="bk")
        bv_t = res.tile([1, DG], F16, tag="bv")
        bo_t = res.tile([1, DG], F16, tag="bo")
        ones_k = res.tile([HD, 1], F16, tag="ones_k")      # [128,1] ones
        ones_r16 = res.tile([1, HD], F16, tag="ones_r16")  # [1,128] ones
        ones_r32 = res.tile([1, HD], F32, tag="ones_r32")
        nc.sync.dma_start(bq_t[:], bq.rearrange("(m p) -> p m", p=HD))
        nc.sync.dma_start(bk_t[:], bk.rearrange("(m p) -> p m", p=HD))
        nc.sync.dma_start(bv_t[:], bv[None, :])
        nc.sync.dma_start(bo_t[:], bo[None, :])
        nc.vector.memset(ones_k[:], 1.0)
        nc.vector.memset(ones_r16[:], 1.0)
        nc.vector.memset(ones_r32[:], 1.0)

        # collective bounce buffers
        bounce_in = []
        bounce_out = []
        for j in range(GH):
            bounce_in.append(dram.tile([HD, GH, NB], F16, tag=f"bi{j}",
                                       name=f"bi{j}"))
            bounce_out.append(dram.tile([4, HD, GH, NB], F16, tag=f"bg{j}",
                                        name=f"bg{j}"))

        with ExitStack() as c12:
            # phase 1+2 residents
            ph = c12.enter_context(tc.tile_pool(name="ph", bufs=1))
            qT = ph.tile([HD, GH, S], F16, tag="qT")        # [128, 4, 2048]
            kTn = ph.tile([HD, GH, S], F16, tag="kTn")
            ckT_t = ph.tile([HD, GH, PC], F16, tag="ckT")
            cv_t = ph.tile([HD, PC // HD, DG], F16, tag="cv")   # [128, 16, 512]
            vn_t = ph.tile([HD, S // HD, DG], F16, tag="vn")
            nc.sync.dma_start(ckT_t[:], ckT.rearrange("(m p) s -> p m s", p=HD))
            nc.sync.dma_start(cv_t[:], cv.rearrange("(ss p) d -> p ss d", p=HD))

            # ---- phase 1: projections ----
            with tc.tile_pool(name="px", bufs=1) as px, \
                 tc.tile_pool(name="pw", bufs=2) as pw, \
                 tc.tile_pool(name="ps1", bufs=1, space="PSUM") as ps1:
                xres = px.tile([HD, NDC, S], F16, tag="xres")   # 8.4 MB
                xr = xT.rearrange("(kc p) s -> p kc s", p=HD)
                for kq in range(4):
                    nc.sync.dma_start(xres[:, 4 * kq:4 * (kq + 1), :],
                                      xr[:, 4 * kq:4 * (kq + 1), :])
                wvt = px.tile([HD, NDC, DG], F16, tag="wvt")    # 2.1 MB
                nc.sync.dma_start(wvt[:],
                                  wv.rearrange("(kc p) n -> p kc n", p=HD))

                # q pass then k pass: weights stay loaded across the 4 s-blocks
                for wsrc, dst, bias_t, scale in (
                        (wq, qT, bq_t, INV_SQRT_HD), (wk, kTn, bk_t, 1.0)):
                    for m in range(GH):
                        wt = pw.tile([HD, NDC, HD], F16, tag="wqk", name="wt")
                        nc.sync.dma_start(
                            wt[:], wsrc[:, HD * m:HD * (m + 1)].rearrange(
                                "(kc p) n -> p kc n", p=HD))
                        psq = [ps1.tile([HD, NB], F32,
                                        tag=f"pp{4 * (m % 2) + sb}",
                                        name=f"psq{sb}") for sb in range(4)]
                        for kc in range(NDC):
                            for sb in range(4):
                                nc.tensor.matmul(
                                    psq[sb][:], wt[:, kc, :],
                                    xres[:, kc, NB * sb:NB * (sb + 1)],
                                    start=(kc == 0), stop=(kc == NDC - 1))
                        for sb in range(4):
                            nc.scalar.activation(
                                dst[:, m, NB * sb:NB * (sb + 1)], psq[sb][:],
                                AF.Identity, bias=bias_t[:, m:m + 1], scale=scale)

                # v pass (natural layout)
                for ss in range(S // HD):
                    psv = ps1.tile([HD, DG], F32, tag=f"pp{ss % 8}", name="psv")
                    for kc in range(NDC):
                        nc.tensor.matmul(psv[:],
                                         xres[:, kc, HD * ss:HD * (ss + 1)],
                                         wvt[:, kc, :],
                                         start=(kc == 0), stop=False)
                    nc.tensor.matmul(psv[:], ones_r16[:], bv_t[:],
                                     start=False, stop=True)
                    nc.any.tensor_copy(vn_t[:, ss, :], psv[:])

            # ---- phase 2: attention per head + AllGather ----
            with tc.tile_pool(name="p2", bufs=6) as p2, \
                 tc.tile_pool(name="zp", bufs=2) as zp, \
                 tc.tile_pool(name="ap", bufs=2) as apool, \
                 tc.tile_pool(name="ps2", bufs=1, space="PSUM") as ps2:
                for j in range(GH):
                    head_scope = nc.named_scope(f"head{j}")
                    head_scope.__enter__()
                    ahead = apool.tile([HD, GH, NB], F16, tag="ah")
                    for sb in range(4):
                        PA = ps2.tile([HD, NB], F32, tag="PA", name="PA")
                        zacc = zp.tile([HD, NB], F16, tag="z")
                        qTs = qT[:, j, NB * sb:NB * (sb + 1)]
                        for c2 in range(NKC // 2):
                            pss = ps2.tile([HD, 2, NB], F32,
                                           tag=f"psS{c2 % 3}", name="pss")
                            e2 = p2.tile([HD, 2, NB], F16, tag="e")
                            for i in range(2):
                                c = 2 * c2 + i
                                if c < PC // HD:
                                    kt = ckT_t[:, j, HD * c:HD * (c + 1)]
                                else:
                                    cc = c - PC // HD
                                    kt = kTn[:, j, HD * cc:HD * (cc + 1)]
                                nc.tensor.matmul(pss[:, i, :], kt, qTs,
                                                 start=True, stop=True)
                            nc.scalar.activation(e2[:], pss[:], AF.Exp)
                            for i in range(2):
                                c = 2 * c2 + i
                                if c < PC // HD:
                                    vt = cv_t[:, c, HD * j:HD * (j + 1)]
                                else:
                                    vt = vn_t[:, c - PC // HD,
                                              HD * j:HD * (j + 1)]
                                nc.tensor.matmul(PA[:], vt, e2[:, i, :],
                                                 start=(c == 0),
                                                 stop=(c == NKC - 1),
                                                 skip_group_check=True)
                            if c2 == 0:
                                nc.vector.tensor_tensor(zacc[:], e2[:, 0, :],
                                                        e2[:, 1, :], ALU.add)
                            else:
                                nc.vector.tensor_tensor(zacc[:], zacc[:],
                                                        e2[:, 0, :], ALU.add)
                                nc.vector.tensor_tensor(zacc[:], zacc[:],
                                                        e2[:, 1, :], ALU.add)
                        psz = ps2.tile([1, NB], F32, tag="psS0", name="psz")
                        nc.tensor.matmul(psz[:], ones_k[:], zacc[:],
                                         start=True, stop=True)
                        zinv = zp.tile([1, NB], F32, tag="zi")
                        nc.vector.reciprocal_approx_fast(zinv[:], psz[:])
                        psb = ps2.tile([HD, NB], F32, tag="psS1", name="psb")
                        nc.tensor.matmul(psb[:], ones_r32[:], zinv[:],
                                         start=True, stop=True)
                        zb = zp.tile([HD, NB], F32, tag="zb")
                        nc.vector.tensor_copy(zb[:], psb[:])
                        nc.vector.tensor_tensor(ahead[:, sb, :], PA[:], zb[:],
                                                ALU.mult)
                    nc.sync.dma_start(bounce_in[j][:], ahead[:])
                    nc.gpsimd.collective_compute(
                        "AllGather", ALU.bypass, replica_groups=GROUPS,
                        ins=[bounce_in[j].opt()], outs=[bounce_out[j].opt()])
                    head_scope.__exit__(None, None, None)

        # ---- phase 3: output projection (full sequence, 512-col wo slice) ----
        with tc.tile_pool(name="p3", bufs=3) as p3, \
             tc.tile_pool(name="lt3", bufs=1) as ltp, \
             tc.tile_pool(name="wo3", bufs=1) as wop, \
             tc.tile_pool(name="ps3", bufs=1, space="PSUM") as ps3:
            wot = wop.tile([HD, 16, NB], F16, tag="wo")
            nc.sync.dma_start(wot[:], wo.rearrange("(c p) n -> p c n", p=HD))
            # one big load per (j, r): [128, 4, 512] contiguous in the bounce
            lts = []
            for j in range(GH):
                for r in range(4):
                    lt = ltp.tile([HD, GH, NB], F16, tag=f"lt{4 * j + r}",
                                  name=f"lt{4 * j + r}")
                    nc.sync.dma_start(lt[:], bounce_out[j][r])
                    lts.append(lt)
            for m in range(S // HD):
                psO = ps3.tile([HD, NB], F32, tag=f"psO{m % 2}", name="psO")
                for jr in range(16):
                    nc.tensor.matmul(
                        psO[:],
                        lts[jr][:, m // 4, HD * (m % 4):HD * (m % 4 + 1)],
                        wot[:, jr, :],
                        start=(jr == 0), stop=False, skip_group_check=True)
                nc.tensor.matmul(psO[:], ones_r16[:], bo_t[:],
                                 start=False, stop=True, skip_group_check=True)
                ot = p3.tile([HD, NB], F32, tag="ot")
                nc.any.tensor_copy(ot[:], psO[:])
                nc.sync.dma_start(y[HD * m:HD * (m + 1), :], ot[:])

    nc.compile()
    return nc


_BUILT = None


def get_built():
    global _BUILT
    if _BUILT is None:
        _BUILT = build()
    return _BUILT


def make_in_maps(x, cache_k, cache_v, wq, bq, wk, bk, wv, bv, wo, bo):
    x = np.asarray(x)
    cache_k = np.asarray(cache_k)
    cache_v = np.asarray(cache_v)
    wq, bq = np.asarray(wq), np.asarray(bq)
    wk, bk = np.asarray(wk), np.asarray(bk)
    wv, bv = np.asarray(wv), np.asarray(bv)
    wo, bo = np.asarray(wo), np.asarray(bo)

    # permute wo rows to match gather order: lhsT chunk jr=(4j+r) holds head 4r+j
    perm = np.concatenate([
        np.arange(HD * (4 * r + j), HD * (4 * r + j) + HD)
        for j in range(GH) for r in range(4)
    ])
    wo_p = wo[perm, :]

    in_maps = []
    for c in range(8):
        b, g = divmod(c, 4)
        sl = slice(DG * g, DG * (g + 1))
        in_maps.append({
            "xT": np.ascontiguousarray(x[b].T).astype(np.float16),
            "wq": wq[:, sl].astype(np.float16),
            "bq": (bq[sl] * INV_SQRT_HD).astype(np.float16),
            "wk": wk[:, sl].astype(np.float16),
            "bk": bk[sl].astype(np.float16),
            "wv": wv[:, sl].astype(np.float16),
            "bv": bv[sl].astype(np.float16),
            "ckT": np.ascontiguousarray(cache_k[b][:, sl].T).astype(np.float16),
            "cv": cache_v[b][:, sl].astype(np.float16),
            "wo": wo_p[:, sl].astype(np.float16),
            "bo": bo[sl].astype(np.float16),
        })
    return in_maps


def assemble(results):
    out = np.empty((B, S, D), np.float32)
    for c in range(8):
        b, g = divmod(c, 4)
        out[b, :, DG * g:DG * (g + 1)] = results[c]["y"]
    return out


def kernel(**inputs):
    nc = get_built()
    in_maps = make_in_maps(**inputs)
    res = run_bass_kernel_spmd(nc, in_maps, core_ids=list(range(8)))
    return assemble(res.results)

